# revision 26
# baseline (speedup 1.0000x reference)
"""Causal self-attention (B=2, T=2048, C=1024, H=16 heads) on 8 trn2 NeuronCores.

Sharding: data-parallel over batch (2) x tensor-parallel over heads (4 groups
of 4 heads) = 8 shards.  Each core computes the QKV projections for its 4
heads, causal attention scores in transposed [k, q] layout (so the exp'd
scores feed the A@V matmul directly -- no on-chip transposes anywhere), the
attention output, and a row-parallel partial of the output projection.

Host side: inputs are pre-transposed per shard (x[b].T, W[rows].T, ...);
outputs are re-assembled here (partial out-proj sums over the 4 head groups
of each batch, attention weights normalized by the returned denominators --
softmax without max-subtraction is safe here since |scores| < ~10).

Device outputs per core:
  u_out  [4, 16, 128, 2048]  u_out[h, kt, p, q] = exp(s[q, 128*kt+p]/8),
                             masked to the causal triangle, unnormalized.
                             Only kt <= q//128 blocks are written; the rest
                             stay zero (PJRT donates zero output buffers).
  den_out [4, 2048]          softmax denominators per (head, q).
  y_out  [2048, 1024]        this head-group's partial of y @ Wo.T (no bias).
"""

import numpy as np

B, T, C, H = 2, 2048, 1024, 16
D = C // H          # 64
NH = 4              # heads per core
HD = NH * D         # 256 local head dim
NCORES = 8

_compiled = None
LAST_RESULTS = None  # BassKernelResults of the most recent run (for test.py)


def _pin_act_table_set():
    """Keep Exp/Ln only in natural_log_exp_and_others so the act-table pass
    never alternates between exp_and_others and the ln set (each switch costs
    ~2.7us on ScalarE).  Set order/indices are preserved."""
    import concourse.bacc as bacc
    import concourse.hw_specs as hw_specs
    import concourse.mybir as mybir
    AF = mybir.ActivationFunctionType
    if getattr(bacc, "_act_tables_pinned", False):
        return
    orig = hw_specs.get_activation_tables

    def pinned(arch):
        tabs = {k: set(v) for k, v in orig(arch).items()}
        for k, v in tabs.items():
            if k != "natural_log_exp_and_others":
                v.discard(AF.Exp)
                v.discard(AF.Ln)
        return tabs

    bacc.get_activation_tables = pinned
    bacc._act_tables_pinned = True


def _build_nc(n_chunks=4, do_qkv=True, do_proj=True, do_udma=True, do_tail=True):
    import concourse.bacc as bacc
    import concourse.tile as tile
    import concourse.mybir as mybir

    _pin_act_table_set()

    f32 = mybir.dt.float32
    f32r = mybir.dt.float32r
    f16 = mybir.dt.float16
    AF = mybir.ActivationFunctionType
    OP = mybir.AluOpType

    nc = bacc.Bacc(None, target_bir_lowering=False, debug=False)

    xT_d = nc.dram_tensor("xT", [C, T], f32r, kind="ExternalInput")
    wq_d = nc.dram_tensor("wqT", [C, HD], f32r, kind="ExternalInput")
    wk_d = nc.dram_tensor("wkT", [C, HD], f32r, kind="ExternalInput")
    wv_d = nc.dram_tensor("wvT", [C, HD], f32r, kind="ExternalInput")
    wo_d = nc.dram_tensor("woT", [HD, C], f32r, kind="ExternalInput")
    bq_d = nc.dram_tensor("bq", [HD], f32, kind="ExternalInput")
    bk_d = nc.dram_tensor("bk", [HD], f32, kind="ExternalInput")
    bv_d = nc.dram_tensor("bv", [HD], f32r, kind="ExternalInput")
    u_d = nc.dram_tensor("u_out", [NH, 4, 128, 16, 512], f16, kind="ExternalOutput")
    rec_d = nc.dram_tensor("rec_out", [NH, T], f32r, kind="ExternalOutput")
    y_d = nc.dram_tensor("y_out", [T, C], f32, kind="ExternalOutput")


    with tile.TileContext(nc) as tc, \
         tc.tile_pool(name="constp", bufs=1) as constp, \
         tc.tile_pool(name="wp", bufs=1) as wp, \
         tc.tile_pool(name="per", bufs=1) as per, \
         tc.tile_pool(name="big", bufs=2) as big, \
         tc.tile_pool(name="stagep", bufs=3) as stagep, \
         tc.tile_pool(name="psA", bufs=2, space="PSUM") as psA, \
         tc.tile_pool(name="psB", bufs=4, space="PSUM") as psB:

        # ---------------- constants ----------------
        ones_f = constp.tile([128, 128], f32)
        nc.gpsimd.memset(ones_f[:], 1.0)
        ones_sb = constp.tile([128, 128], f32r)
        nc.scalar.activation(ones_sb[:], ones_f[:], AF.Identity)
        # trimask[p, y] = 1.0 where p <= y else 0.0  (causal keep-mask for a
        # diagonal 128x128 block in [k, q] layout)
        trimask = constp.tile([128, 128], f32)
        nc.gpsimd.memset(trimask[:], 1.0)
        nc.gpsimd.affine_select(
            out=trimask[:], in_=trimask[:],
            compare_op=OP.is_ge, fill=0.0, base=0,
            pattern=[[1, 128]], channel_multiplier=-1)

        bq_sb = constp.tile([128, 2], f32)
        nc.sync.dma_start(bq_sb[:], bq_d[:].rearrange("(m p) -> p m", p=128))
        bk_sb = constp.tile([128, 2], f32)
        nc.sync.dma_start(bk_sb[:], bk_d[:].rearrange("(m p) -> p m", p=128))
        bv_row = constp.tile([1, HD], f32r)
        nc.sync.dma_start(bv_row[0:1, :], bv_d[:].rearrange("(o x) -> o x", o=1))

        # ---------------- weights + x^T ----------------
        wq_sb = wp.tile([128, 8, HD], f32r)
        nc.sync.dma_start(wq_sb[:], wq_d[:].rearrange("(o p) n -> p o n", p=128))
        wk_sb = wp.tile([128, 8, HD], f32r)
        nc.sync.dma_start(wk_sb[:], wk_d[:].rearrange("(o p) n -> p o n", p=128))
        wv_sb = wp.tile([128, 8, HD], f32r)
        nc.sync.dma_start(wv_sb[:], wv_d[:].rearrange("(o p) n -> p o n", p=128))
        wo_sb = wp.tile([128, 2, C], f32r)
        nc.sync.dma_start(wo_sb[:], wo_d[:].rearrange("(o p) n -> p o n", p=128))

        xA = big.tile([128, 4, T], f32r, tag="big", name="xA")
        xB = big.tile([128, 4, T], f32r, tag="big", name="xB")
        for ko in range(8):
            dst = xA if ko < 4 else xB
            nc.sync.dma_start(dst[:, ko % 4, :], xT_d[128 * ko:128 * (ko + 1), :])

        # broadcast bv across partitions via a rank-1 matmul
        bv_bc = constp.tile([128, HD], f32)
        ps_bv = psB.tile([128, HD], f32, tag="psB", name="ps_bv")
        nc.tensor.matmul(ps_bv[:], ones_sb[0:1, :], bv_row[0:1, :],
                         start=True, stop=True)
        nc.vector.tensor_copy(bv_bc[:], ps_bv[:])

        # ---------------- QKV projections ----------------
        # QT/KT layout [128, 2, T]: row p of block m holds head-dim 128*m+p.
        QT = per.tile([128, 2, T], f32r)
        KT = per.tile([128, 2, T], f32r)
        for w_sb, out_sb, b_sb in (((wq_sb, QT, bq_sb), (wk_sb, KT, bk_sb)) if do_qkv else ()):
            for m in range(2):
                pss = [psB.tile([128, 512], f32, tag="psB",
                                name=f"ps_qk_{m}_{c4}") for c4 in range(4)]
                for ko in range(8):
                    xt = xA if ko < 4 else xB
                    for c4 in range(4):
                        nc.tensor.matmul(
                            pss[c4][:],
                            w_sb[:, ko, 128 * m:128 * (m + 1)],
                            xt[:, ko % 4, 512 * c4:512 * (c4 + 1)],
                            start=(ko == 0), stop=(ko == 7))
                for c4 in range(4):
                    nc.scalar.activation(
                        out_sb[:, m, 512 * c4:512 * (c4 + 1)], pss[c4][:],
                        AF.Identity, bias=b_sb[:, m:m + 1], scale=1.0)

        # V in natural [t, head-dim] layout, augmented with a ones column per
        # head so the A@V matmul also produces softmax denominators.
        # Per (t, pair): cols 0-63 = V_even, col 64 = 1.0 (den -> psum part 64);
        # col 97 = 1.0 and cols 129-192 = V_odd, used via the AP [65:193] so
        # the odd head's den lands on psum partition 32 and Y on 64-127.
        # Cols 65-96 and 98-128 are zeroed (they feed ignored psum partitions
        # but must not be NaN).
        vbuf = per.tile([128, 16, 2, 196], f16)
        zeros_f = constp.tile([128, 64], f32)
        nc.gpsimd.memset(zeros_f[:], 0.0)
        for t in range(16):
            nc.vector.tensor_copy(vbuf[:, t, :, 64:65], ones_f[:, 0:2]
                                  .rearrange("p (a b) -> p a b", a=2))
            nc.vector.tensor_copy(vbuf[:, t, :, 97:98], ones_f[:, 0:2]
                                  .rearrange("p (a b) -> p a b", a=2))
            nc.vector.tensor_copy(vbuf[:, t, :, 65:97], zeros_f[:, 0:64]
                                  .rearrange("p (a b) -> p a b", a=2))
            nc.vector.tensor_copy(vbuf[:, t, :, 98:129], zeros_f[:, 0:62]
                                  .rearrange("p (a b) -> p a b", a=2))
        def emit_v_tile(t):
            psv = psB.tile([128, HD], f32, tag="psB", name=f"ps_v_{t}")
            for ko in range(8):
                xt = xA if ko < 4 else xB
                nc.tensor.matmul(
                    psv[:],
                    xt[:, ko % 4, 128 * t:128 * (t + 1)],
                    wv_sb[:, ko, :],
                    start=(ko == 0), stop=(ko == 7))
            psv4 = psv[:].rearrange("p (a b x) -> p a b x", b=2, x=64)
            bv4 = bv_bc[:].rearrange("p (a b x) -> p a b x", b=2, x=64)
            nc.vector.tensor_tensor(vbuf[:, t, :, 0:64],
                                    psv4[:, :, 0, :], bv4[:, :, 0, :], OP.add)
            nc.vector.tensor_tensor(vbuf[:, t, :, 129:193],
                                    psv4[:, :, 1, :], bv4[:, :, 1, :], OP.add)

        # ---------------- attention ----------------
        # YTu[p, blk, t] = (normalized) Y^T at head-dim 128*blk+p
        YTu = per.tile([128, 2, T], f32r)
        for c in range(n_chunks):           # query chunk of 512
            # V tiles for this chunk's new k range (emitted late so the PE
            # produces early score tiles first and ScalarE exp starts early)
            if do_qkv:
                for t in range(4 * c, 4 * c + 4):
                    emit_v_tile(t)
            for h in range(4):       # local head
                mb = h // 2          # block in QT/KT; also the vbuf pair idx
                pb = (h % 2) * 64    # partition base of this head's 64 dims
                hb = slice(pb, pb + 64)
                nkt = 4 * c + 4      # live k tiles for this query chunk
                U = big.tile([128, 16, 512], f16, tag="u", bufs=2, name=f"U_{c}_{h}")
                # scores^T = K^T.T-weighted streams of Q^T, then exp
                for g2 in range(nkt // 2):
                    psS = psA.tile([128, 2, 512], f32, tag="psA",
                                   name=f"psS_{c}_{h}_{g2}")
                    for d2 in range(2):
                        kt = 2 * g2 + d2
                        nc.tensor.matmul(
                            psS[:, d2, :],
                            KT[hb, mb, 128 * kt:128 * (kt + 1)],
                            QT[hb, mb, 512 * c:512 * (c + 1)],
                            start=True, stop=True)
                    nc.scalar.activation(U[:, 2 * g2:2 * g2 + 2, :], psS[:],
                                         AF.Exp, scale=0.125)
                # causal mask on the 4 diagonal blocks of this chunk
                for d in range(4):
                    kt = 4 * c + d
                    if d > 0:
                        nc.gpsimd.memset(U[:, kt, 0:128 * d], 0.0)
                    nc.gpsimd.tensor_tensor(
                        U[:, kt, 128 * d:128 * (d + 1)],
                        U[:, kt, 128 * d:128 * (d + 1)], trimask[:], OP.mult)
                # unnormalized probabilities out (transposed blocks)
                if do_udma:
                    nc.sync.dma_start(u_d[h, c, :, 0:nkt, :], U[:, 0:nkt, :])
                # Y^T (+ denominator row) = sum_k V_aug^T[k,:] U[k,:]
                psY = psB.tile([128, 512], f32, tag="psB", name=f"psY_{c}_{h}")
                for kt in range(nkt):
                    if h % 2 == 0:
                        lhsv, yout = vbuf[:, kt, mb, 0:65], psY[0:65, :]
                    else:
                        lhsv, yout = vbuf[:, kt, mb, 65:193], psY[0:128, :]
                    nc.tensor.matmul(yout, lhsv, U[:, kt, :],
                                     start=(kt == 0), stop=(kt == nkt - 1))
                if not do_tail:
                    nc.scalar.copy(YTu[hb.start:hb.stop, mb, 512 * c:512 * (c + 1)],
                                   psY[pb:pb + 64, :])
                    continue
                dr = 64 if h % 2 == 0 else 32   # denominator psum partition
                # reciprocal of the denominator row: exp(-ln(d)) on ACT
                rec = stagep.tile([128, 512], f32r, tag="stg", name=f"rec_{c}_{h}", bufs=3)
                nc.scalar.activation(rec[dr:dr + 1, :], psY[dr:dr + 1, :], AF.Ln)
                nc.scalar.activation(rec[dr:dr + 1, :], rec[dr:dr + 1, :],
                                     AF.Exp, scale=-1.0)
                nc.sync.dma_start(rec_d[h:h + 1, 512 * c:512 * (c + 1)],
                                  rec[dr:dr + 1, :])
                # copy unnormalized Y^T rows, broadcast recip, normalize
                nc.any.tensor_copy(YTu[hb.start:hb.stop, mb, 512 * c:512 * (c + 1)],
                                   psY[pb:pb + 64, :])
                psb = psB.tile([128, 512], f32, tag="psB", name=f"psb_{c}_{h}")
                nc.tensor.matmul(psb[:], ones_sb[dr:dr + 1, :], rec[dr:dr + 1, :],
                                 start=True, stop=True)
                nc.vector.tensor_tensor(
                    YTu[hb.start:hb.stop, mb, 512 * c:512 * (c + 1)],
                    YTu[hb.start:hb.stop, mb, 512 * c:512 * (c + 1)],
                    psb[pb:pb + 64, :], OP.mult)
            # out-projection for this chunk's 4 t-tiles (all its heads done)
            for t in (range(4 * c, 4 * c + 4) if do_proj else []):
                ysb = stagep.tile([128, 1024], f32, tag="ysb",
                                  name=f"ysb_{t}", bufs=2)
                for cb in range(2):
                    psP = psB.tile([128, 512], f32, tag="psB",
                                   name=f"psP_{t}_{cb}")
                    for blk in range(2):
                        nc.tensor.matmul(
                            psP[:],
                            YTu[:, blk, 128 * t:128 * (t + 1)],
                            wo_sb[:, blk, 512 * cb:512 * (cb + 1)],
                            start=(blk == 0), stop=(blk == 1))
                    nc.any.tensor_copy(ysb[:, 512 * cb:512 * (cb + 1)], psP[:])
                nc.sync.dma_start(y_d[128 * t:128 * (t + 1), :], ysb[:])

    nc.compile()
    return nc


def _get_nc():
    global _compiled
    if _compiled is None:
        _compiled = _build_nc()
    return _compiled


def kernel(x, Wq, bq, Wk, bk, Wv, bv, Wo, bo):
    global LAST_RESULTS
    from concourse.bass_utils import run_bass_kernel_spmd

    x = np.asarray(x, np.float32)
    Wq, Wk, Wv, Wo = (np.asarray(a, np.float32) for a in (Wq, Wk, Wv, Wo))
    bq, bk, bv, bo = (np.asarray(a, np.float32) for a in (bq, bk, bv, bo))

    nc = _get_nc()
    in_maps = []
    for core in range(NCORES):
        b, g = divmod(core, NCORES // B)
        hs = slice(HD * g, HD * (g + 1))
        in_maps.append({
            "xT": np.ascontiguousarray(x[b].T),
            "wqT": np.ascontiguousarray(Wq[hs, :].T),
            "wkT": np.ascontiguousarray(Wk[hs, :].T),
            "wvT": np.ascontiguousarray(Wv[hs, :].T),
            "woT": np.ascontiguousarray(Wo[:, hs].T),
            "bq": np.ascontiguousarray(bq[hs]),
            "bk": np.ascontiguousarray(bk[hs]),
            "bv": np.ascontiguousarray(bv[hs]),
        })

    LAST_RESULTS = run_bass_kernel_spmd(nc, in_maps, core_ids=list(range(NCORES)))
    results = LAST_RESULTS.results

    y = np.zeros((B, T, C), np.float32)
    att = np.zeros((B, H, T, T), np.float32)
    for core, res in enumerate(results):
        b, g = divmod(core, NCORES // B)
        y[b] += res["y_out"]
        rec = res["rec_out"]          # [4, T] = 1/denominator
        U = res["u_out"]              # [4, 16, 128, T]
        for lh in range(NH):
            Hh = NH * g + lh
            # u[lh]: [4(c), 128(p), 16(kt), 512(q)] -> [k, q]
            W = U[lh].transpose(2, 1, 0, 3).reshape(T, T)
            np.multiply(W.T, rec[lh][:, None], out=att[b, Hh])
    y += bo
    return y, att


# revision 32
# speedup vs baseline: 1.0073x; 1.0073x over previous
"""Causal self-attention (B=2, T=2048, C=1024, H=16 heads) on 8 trn2 NeuronCores.

Sharding: data-parallel over batch (2) x tensor-parallel over heads (4 groups
of 4 heads) = 8 shards.  Each core computes the QKV projections for its 4
heads, causal attention scores in transposed [k, q] layout (so the exp'd
scores feed the A@V matmul directly -- no on-chip transposes anywhere), the
attention output, and a row-parallel partial of the output projection.

Host side: inputs are pre-transposed per shard (x[b].T, W[rows].T, ...);
outputs are re-assembled here (partial out-proj sums over the 4 head groups
of each batch, attention weights normalized by the returned denominators --
softmax without max-subtraction is safe here since |scores| < ~10).

Device outputs per core:
  u_out  [4, 16, 128, 2048]  u_out[h, kt, p, q] = exp(s[q, 128*kt+p]/8),
                             masked to the causal triangle, unnormalized.
                             Only kt <= q//128 blocks are written; the rest
                             stay zero (PJRT donates zero output buffers).
  den_out [4, 2048]          softmax denominators per (head, q).
  y_out  [2048, 1024]        this head-group's partial of y @ Wo.T (no bias).
"""

import numpy as np

B, T, C, H = 2, 2048, 1024, 16
D = C // H          # 64
NH = 4              # heads per core
HD = NH * D         # 256 local head dim
NCORES = 8

_compiled = None
LAST_RESULTS = None  # BassKernelResults of the most recent run (for test.py)


def _pin_act_table_set():
    """Keep Exp/Ln only in natural_log_exp_and_others so the act-table pass
    never alternates between exp_and_others and the ln set (each switch costs
    ~2.7us on ScalarE).  Set order/indices are preserved."""
    import concourse.bacc as bacc
    import concourse.hw_specs as hw_specs
    import concourse.mybir as mybir
    AF = mybir.ActivationFunctionType
    if getattr(bacc, "_act_tables_pinned", False):
        return
    orig = hw_specs.get_activation_tables

    def pinned(arch):
        tabs = {k: set(v) for k, v in orig(arch).items()}
        for k, v in tabs.items():
            if k != "natural_log_exp_and_others":
                v.discard(AF.Exp)
                v.discard(AF.Ln)
        return tabs

    bacc.get_activation_tables = pinned
    bacc._act_tables_pinned = True


def _build_nc(n_chunks=4, do_qkv=True, do_proj=True, do_udma=True, do_tail=True):
    import concourse.bacc as bacc
    import concourse.tile as tile
    import concourse.mybir as mybir

    _pin_act_table_set()

    f32 = mybir.dt.float32
    f32r = mybir.dt.float32r
    f16 = mybir.dt.float16
    AF = mybir.ActivationFunctionType
    OP = mybir.AluOpType

    nc = bacc.Bacc(None, target_bir_lowering=False, debug=False)

    xT_d = nc.dram_tensor("xT", [C, T], f32r, kind="ExternalInput")
    wq_d = nc.dram_tensor("wqT", [C, HD], f32r, kind="ExternalInput")
    wk_d = nc.dram_tensor("wkT", [C, HD], f32r, kind="ExternalInput")
    wv_d = nc.dram_tensor("wvT", [C, HD], f32r, kind="ExternalInput")
    wo_d = nc.dram_tensor("woT", [HD, C], f32r, kind="ExternalInput")
    bq_d = nc.dram_tensor("bq", [HD], f32, kind="ExternalInput")
    bk_d = nc.dram_tensor("bk", [HD], f32, kind="ExternalInput")
    bv_d = nc.dram_tensor("bv", [HD], f32r, kind="ExternalInput")
    u_d = nc.dram_tensor("u_out", [NH, 4, 128, 16, 512], f16, kind="ExternalOutput")
    rec_d = nc.dram_tensor("rec_out", [NH, T], f32r, kind="ExternalOutput")
    y_d = nc.dram_tensor("y_out", [T, C], f32, kind="ExternalOutput")


    with tile.TileContext(nc) as tc, \
         tc.tile_pool(name="constp", bufs=1) as constp, \
         tc.tile_pool(name="wp", bufs=1) as wp, \
         tc.tile_pool(name="per", bufs=1) as per, \
         tc.tile_pool(name="big", bufs=2) as big, \
         tc.tile_pool(name="stagep", bufs=3) as stagep, \
         tc.tile_pool(name="psA", bufs=2, space="PSUM") as psA, \
         tc.tile_pool(name="psB", bufs=3, space="PSUM") as psB, \
         tc.tile_pool(name="psC", bufs=1, space="PSUM") as psC:

        # ---------------- constants ----------------
        ones_f = constp.tile([128, 128], f32)
        nc.gpsimd.memset(ones_f[:], 1.0)
        ones_sb = constp.tile([128, 128], f32r)
        nc.scalar.activation(ones_sb[:], ones_f[:], AF.Identity)
        # trimask[p, y] = 1.0 where p <= y else 0.0  (causal keep-mask for a
        # diagonal 128x128 block in [k, q] layout)
        trimask = constp.tile([128, 128], f32)
        nc.gpsimd.memset(trimask[:], 1.0)
        nc.gpsimd.affine_select(
            out=trimask[:], in_=trimask[:],
            compare_op=OP.is_ge, fill=0.0, base=0,
            pattern=[[1, 128]], channel_multiplier=-1)

        bq_sb = constp.tile([128, 2], f32)
        nc.sync.dma_start(bq_sb[:], bq_d[:].rearrange("(m p) -> p m", p=128))
        bk_sb = constp.tile([128, 2], f32)
        nc.sync.dma_start(bk_sb[:], bk_d[:].rearrange("(m p) -> p m", p=128))
        bv_row = constp.tile([1, HD], f32r)
        nc.sync.dma_start(bv_row[0:1, :], bv_d[:].rearrange("(o x) -> o x", o=1))

        # ---------------- weights + x^T ----------------
        wq_sb = wp.tile([128, 8, HD], f32r)
        nc.sync.dma_start(wq_sb[:], wq_d[:].rearrange("(o p) n -> p o n", p=128))
        wk_sb = wp.tile([128, 8, HD], f32r)
        nc.sync.dma_start(wk_sb[:], wk_d[:].rearrange("(o p) n -> p o n", p=128))
        wv_sb = wp.tile([128, 8, HD], f32r)
        nc.sync.dma_start(wv_sb[:], wv_d[:].rearrange("(o p) n -> p o n", p=128))
        wo_sb = wp.tile([128, 2, C], f32r)
        nc.sync.dma_start(wo_sb[:], wo_d[:].rearrange("(o p) n -> p o n", p=128))

        xA = big.tile([128, 4, T], f32r, tag="big", name="xA")
        xB = big.tile([128, 4, T], f32r, tag="big", name="xB")
        for ko in range(8):
            dst = xA if ko < 4 else xB
            nc.sync.dma_start(dst[:, ko % 4, :], xT_d[128 * ko:128 * (ko + 1), :])

        # broadcast bv across partitions via a rank-1 matmul
        bv_bc = constp.tile([128, HD], f32)
        ps_bv = psB.tile([128, HD], f32, tag="psB", name="ps_bv")
        nc.tensor.matmul(ps_bv[:], ones_sb[0:1, :], bv_row[0:1, :],
                         start=True, stop=True)
        nc.vector.tensor_copy(bv_bc[:], ps_bv[:])

        # ---------------- QKV projections ----------------
        # QT/KT layout [128, 2, T]: row p of block m holds head-dim 128*m+p.
        QT = per.tile([128, 2, T], f32r)
        KT = per.tile([128, 2, T], f32r)
        for w_sb, out_sb, b_sb in (((wq_sb, QT, bq_sb), (wk_sb, KT, bk_sb)) if do_qkv else ()):
            for m in range(2):
                for cp in range(2):
                    pss = [psB.tile([128, 512], f32, tag="psB",
                                    name=f"ps_qk_{m}_{cp}_{i}") for i in range(2)]
                    for ko in range(8):
                        xt = xA if ko < 4 else xB
                        for i in range(2):
                            c4 = 2 * cp + i
                            nc.tensor.matmul(
                                pss[i][:],
                                w_sb[:, ko, 128 * m:128 * (m + 1)],
                                xt[:, ko % 4, 512 * c4:512 * (c4 + 1)],
                                start=(ko == 0), stop=(ko == 7))
                    for i in range(2):
                        c4 = 2 * cp + i
                        nc.scalar.activation(
                            out_sb[:, m, 512 * c4:512 * (c4 + 1)], pss[i][:],
                            AF.Identity, bias=b_sb[:, m:m + 1], scale=1.0)

        # V in natural [t, head-dim] layout, augmented with a ones column per
        # head so the A@V matmul also produces softmax denominators.
        # Per (t, pair): cols 0-63 = V_even, col 64 = 1.0 (den -> psum part 64);
        # col 97 = 1.0 and cols 129-192 = V_odd, used via the AP [65:193] so
        # the odd head's den lands on psum partition 32 and Y on 64-127.
        # Cols 65-96 and 98-128 are zeroed (they feed ignored psum partitions
        # but must not be NaN).
        vbuf = per.tile([128, 16, 2, 196], f16)
        zeros_f = constp.tile([128, 64], f32)
        nc.gpsimd.memset(zeros_f[:], 0.0)
        for t in range(16):
            nc.vector.tensor_copy(vbuf[:, t, :, 64:65], ones_f[:, 0:2]
                                  .rearrange("p (a b) -> p a b", a=2))
            nc.vector.tensor_copy(vbuf[:, t, :, 97:98], ones_f[:, 0:2]
                                  .rearrange("p (a b) -> p a b", a=2))
            nc.vector.tensor_copy(vbuf[:, t, :, 65:97], zeros_f[:, 0:64]
                                  .rearrange("p (a b) -> p a b", a=2))
            nc.vector.tensor_copy(vbuf[:, t, :, 98:129], zeros_f[:, 0:62]
                                  .rearrange("p (a b) -> p a b", a=2))
        def emit_v_tile(t):
            psv = psB.tile([128, HD], f32, tag="psB", name=f"ps_v_{t}")
            for ko in range(8):
                xt = xA if ko < 4 else xB
                nc.tensor.matmul(
                    psv[:],
                    xt[:, ko % 4, 128 * t:128 * (t + 1)],
                    wv_sb[:, ko, :],
                    start=(ko == 0), stop=(ko == 7))
            psv4 = psv[:].rearrange("p (a b x) -> p a b x", b=2, x=64)
            bv4 = bv_bc[:].rearrange("p (a b x) -> p a b x", b=2, x=64)
            nc.vector.tensor_tensor(vbuf[:, t, :, 0:64],
                                    psv4[:, :, 0, :], bv4[:, :, 0, :], OP.add)
            nc.vector.tensor_tensor(vbuf[:, t, :, 129:193],
                                    psv4[:, :, 1, :], bv4[:, :, 1, :], OP.add)

        # ---------------- attention ----------------
        # YTu[p, blk, t] = (normalized) Y^T at head-dim 128*blk+p
        YTu = per.tile([128, 2, T], f32r)
        for c in range(n_chunks):           # query chunk of 512
            # V tiles for this chunk's new k range (emitted late so the PE
            # produces early score tiles first and ScalarE exp starts early)
            if do_qkv:
                for t in range(4 * c, 4 * c + 4):
                    emit_v_tile(t)
            for h in range(4):       # local head
                mb = h // 2          # block in QT/KT; also the vbuf pair idx
                pb = (h % 2) * 64    # partition base of this head's 64 dims
                hb = slice(pb, pb + 64)
                nkt = 4 * c + 4      # live k tiles for this query chunk
                U = big.tile([128, 16, 512], f16, tag="u", bufs=2, name=f"U_{c}_{h}")
                # scores^T = K^T.T-weighted streams of Q^T, then exp
                for g2 in range(nkt // 2):
                    psS = psA.tile([128, 2, 512], f32, tag="psA",
                                   name=f"psS_{c}_{h}_{g2}")
                    for d2 in range(2):
                        kt = 2 * g2 + d2
                        nc.tensor.matmul(
                            psS[:, d2, :],
                            KT[hb, mb, 128 * kt:128 * (kt + 1)],
                            QT[hb, mb, 512 * c:512 * (c + 1)],
                            start=True, stop=True)
                    nc.scalar.activation(U[:, 2 * g2:2 * g2 + 2, :], psS[:],
                                         AF.Exp, scale=0.125)
                # causal mask on the 4 diagonal blocks of this chunk
                for d in range(4):
                    kt = 4 * c + d
                    if d > 0:
                        nc.gpsimd.memset(U[:, kt, 0:128 * d], 0.0)
                    nc.gpsimd.tensor_tensor(
                        U[:, kt, 128 * d:128 * (d + 1)],
                        U[:, kt, 128 * d:128 * (d + 1)], trimask[:], OP.mult)
                # unnormalized probabilities out (transposed blocks)
                if do_udma:
                    nc.sync.dma_start(u_d[h, c, :, 0:nkt, :], U[:, 0:nkt, :])
                # Y^T (+ denominator row) = sum_k V_aug^T[k,:] U[k,:]
                psY = psB.tile([128, 512], f32, tag="psB", name=f"psY_{c}_{h}")
                for kt in range(nkt):
                    if h % 2 == 0:
                        lhsv, yout = vbuf[:, kt, mb, 0:65], psY[0:65, :]
                    else:
                        lhsv, yout = vbuf[:, kt, mb, 65:193], psY[0:128, :]
                    nc.tensor.matmul(yout, lhsv, U[:, kt, :],
                                     start=(kt == 0), stop=(kt == nkt - 1))
                if not do_tail:
                    nc.scalar.copy(YTu[hb.start:hb.stop, mb, 512 * c:512 * (c + 1)],
                                   psY[pb:pb + 64, :])
                    continue
                dr = 64 if h % 2 == 0 else 32   # denominator psum partition
                # reciprocal of the denominator row: exp(-ln(d)) on ACT
                rec = stagep.tile([128, 512], f32r, tag="stg", name=f"rec_{c}_{h}", bufs=3)
                nc.scalar.activation(rec[dr:dr + 1, :], psY[dr:dr + 1, :], AF.Ln)
                nc.scalar.activation(rec[dr:dr + 1, :], rec[dr:dr + 1, :],
                                     AF.Exp, scale=-1.0)
                nc.sync.dma_start(rec_d[h:h + 1, 512 * c:512 * (c + 1)],
                                  rec[dr:dr + 1, :])
                # copy unnormalized Y^T rows, broadcast recip, normalize
                nc.any.tensor_copy(YTu[hb.start:hb.stop, mb, 512 * c:512 * (c + 1)],
                                   psY[pb:pb + 64, :])
                psb = psC.tile([128, 512], f32, tag="psC", name=f"psb_{c}_{h}")
                nc.tensor.matmul(psb[:], ones_sb[dr:dr + 1, :], rec[dr:dr + 1, :],
                                 start=True, stop=True)
                nc.vector.tensor_tensor(
                    YTu[hb.start:hb.stop, mb, 512 * c:512 * (c + 1)],
                    YTu[hb.start:hb.stop, mb, 512 * c:512 * (c + 1)],
                    psb[pb:pb + 64, :], OP.mult)
            # out-projection for this chunk's 4 t-tiles (all its heads done)
            for t in (range(4 * c, 4 * c + 4) if do_proj else []):
                ysb = stagep.tile([128, 1024], f32, tag="ysb",
                                  name=f"ysb_{t}", bufs=2)
                for cb in range(2):
                    psP = psB.tile([128, 512], f32, tag="psB",
                                   name=f"psP_{t}_{cb}")
                    for blk in range(2):
                        nc.tensor.matmul(
                            psP[:],
                            YTu[:, blk, 128 * t:128 * (t + 1)],
                            wo_sb[:, blk, 512 * cb:512 * (cb + 1)],
                            start=(blk == 0), stop=(blk == 1))
                    nc.any.tensor_copy(ysb[:, 512 * cb:512 * (cb + 1)], psP[:])
                nc.sync.dma_start(y_d[128 * t:128 * (t + 1), :], ysb[:])

    nc.compile()
    return nc


def _get_nc():
    global _compiled
    if _compiled is None:
        _compiled = _build_nc()
    return _compiled


def kernel(x, Wq, bq, Wk, bk, Wv, bv, Wo, bo):
    global LAST_RESULTS
    from concourse.bass_utils import run_bass_kernel_spmd

    x = np.asarray(x, np.float32)
    Wq, Wk, Wv, Wo = (np.asarray(a, np.float32) for a in (Wq, Wk, Wv, Wo))
    bq, bk, bv, bo = (np.asarray(a, np.float32) for a in (bq, bk, bv, bo))

    nc = _get_nc()
    in_maps = []
    for core in range(NCORES):
        b, g = divmod(core, NCORES // B)
        hs = slice(HD * g, HD * (g + 1))
        in_maps.append({
            "xT": np.ascontiguousarray(x[b].T),
            "wqT": np.ascontiguousarray(Wq[hs, :].T),
            "wkT": np.ascontiguousarray(Wk[hs, :].T),
            "wvT": np.ascontiguousarray(Wv[hs, :].T),
            "woT": np.ascontiguousarray(Wo[:, hs].T),
            "bq": np.ascontiguousarray(bq[hs]),
            "bk": np.ascontiguousarray(bk[hs]),
            "bv": np.ascontiguousarray(bv[hs]),
        })

    LAST_RESULTS = run_bass_kernel_spmd(nc, in_maps, core_ids=list(range(NCORES)))
    results = LAST_RESULTS.results

    y = np.zeros((B, T, C), np.float32)
    att = np.zeros((B, H, T, T), np.float32)
    for core, res in enumerate(results):
        b, g = divmod(core, NCORES // B)
        y[b] += res["y_out"]
        rec = res["rec_out"]          # [4, T] = 1/denominator
        U = res["u_out"]              # [4, 16, 128, T]
        for lh in range(NH):
            Hh = NH * g + lh
            # u[lh]: [4(c), 128(p), 16(kt), 512(q)] -> [k, q]
            W = U[lh].transpose(2, 1, 0, 3).reshape(T, T)
            np.multiply(W.T, rec[lh][:, None], out=att[b, Hh])
    y += bo
    return y, att
